# revision 9
# baseline (speedup 1.0000x reference)
"""Trainium2 Bass kernel for nn_Attention_84756884619871.

Causal multi-head attention (B=2, S=2048, D=2048, H=16, Dh=128) with RoPE,
fused QKV projection and output projection.

Sharding (8 NeuronCores): data-parallel over batch (2 groups) x
tensor-parallel over heads (4 cores/group, 4 heads each).  Each core:
  - computes q^T,k^T (transposed layout, RoPE applied) and v for its heads
  - flash-style attention in score-transposed space (p^T[t,s]); softmax
    denominator via ones-vector matmul; no max-subtraction (scores are
    small: exp is safe in fp32)
  - normalized ctx^T shards are AllGather'd over the 4-core group (split in
    two halves along s so the first AG overlaps the second half's compute)
  - each core computes a 512-column slice of the output projection
All matmuls run in float32r (~1.5e-4 rel err, full PE rate at N>=256).
Host assembles the full [2,2048,2048] output from the 8 column slices.
"""

import numpy as np

import concourse.bass as bass
import concourse.tile as tile
import concourse.mybir as mybir
from concourse import bacc
from contextlib import ExitStack

F32 = mybir.dt.float32
F32R = mybir.dt.float32r
AF = mybir.ActivationFunctionType

D = 2048
S = 2048
NCORES = 8
TPDEG = 4          # tensor-parallel group size (heads)
HLOC = 4           # heads per core
DH = 128
SCALE = float(1.0 / np.sqrt(DH))

_STATE: dict = {}


def _chunks(c0):
    """Split columns [c0, 1024) into <=512 pieces, each >=256 when possible."""
    if c0 == 0:
        return [(0, 512), (512, 1024)]
    if c0 <= 256:
        return [(c0, 512), (512, 1024)]
    if c0 == 384:
        return [(384, 512), (512, 1024)]
    if c0 in (512, 640, 768, 896):
        return [(c0, 1024)]
    raise ValueError(c0)


def _build():
    nc = bacc.Bacc("TRN2", target_bir_lowering=False, debug=False, num_devices=NCORES)
    xT = nc.dram_tensor("xT", [D, S], F32, kind="ExternalInput")
    wqk = nc.dram_tensor("wqk", [D, 1024], F32, kind="ExternalInput")
    wv = nc.dram_tensor("wv", [D, 512], F32, kind="ExternalInput")
    wo = nc.dram_tensor("wo", [D, 512], F32, kind="ExternalInput")
    cosT = nc.dram_tensor("cosT", [128, S], F32, kind="ExternalInput")
    sinTs = nc.dram_tensor("sinTs", [128, S], F32, kind="ExternalInput")
    trimask = nc.dram_tensor("trimask", [128, 128], F32, kind="ExternalInput")
    out = nc.dram_tensor("out", [S, 512], F32, kind="ExternalOutput")

    xT3 = xT.ap().rearrange("(ko ki) s -> ki ko s", ki=128).bitcast(F32R)
    wqk3 = wqk.ap().rearrange("(ko ki) c -> ki ko c", ki=128).bitcast(F32R)
    wv3 = wv.ap().rearrange("(ko ki) c -> ki ko c", ki=128).bitcast(F32R)
    wo3 = wo.ap().rearrange("(ko ki) c -> ki ko c", ki=128).bitcast(F32R)

    with tile.TileContext(nc) as tc, ExitStack() as top:
        # ---- persistent tensors -------------------------------------------
        per = top.enter_context(tc.tile_pool(name="persist", bufs=1))
        mask_sb = per.tile([128, 128], F32R, name="mask")
        nc.sync.dma_start(mask_sb[:], trimask.ap().bitcast(F32R))
        nc.gpsimd.tensor_copy(mask_sb[:], mask_sb[:])
        ones_c0 = per.tile([128, 1], F32, name="ones_c0")
        nc.vector.memset(ones_c0[:], 1.0)
        ones_col = per.tile([128, 1], F32R, name="ones_col")
        nc.vector.tensor_copy(ones_col[:], ones_c0[:])
        ones_r0 = per.tile([1, 128], F32, name="ones_r0")
        nc.vector.memset(ones_r0[:], 1.0)
        ones_row = per.tile([1, 128], F32R, name="ones_row")
        nc.vector.tensor_copy(ones_row[:], ones_r0[:])

        dram = top.enter_context(tc.tile_pool(name="dram", bufs=1, space="DRAM"))
        agin = [dram.tile([HLOC * 128, 1024], F32R, name=f"agin{sb}") for sb in range(2)]
        agout = [dram.tile([D, 1024], F32R, name=f"agout{sb}") for sb in range(2)]

        ab = top.enter_context(ExitStack())
        qk_pool = ab.enter_context(tc.tile_pool(name="qkpool", bufs=1))
        qrot = [qk_pool.tile([128, S], F32R, name=f"qrot{h}") for h in range(HLOC)]
        krot = [qk_pool.tile([128, S], F32R, name=f"krot{h}") for h in range(HLOC)]

        # ---- phase A1: q^T,k^T projection + RoPE --------------------------
        with ExitStack() as st:
            wqk_pool = st.enter_context(tc.tile_pool(name="wqkp", bufs=1))
            wqk_sb = wqk_pool.tile([128, 16, 1024], F32R, name="wqk_sb")
            for g in range(4):
                nc.sync.dma_start(wqk_sb[:, 4 * g:4 * g + 4, :],
                                  wqk3[:, 4 * g:4 * g + 4, :])
                nc.gpsimd.tensor_copy(wqk_sb[:, 4 * g:4 * g + 4, :],
                                 wqk_sb[:, 4 * g:4 * g + 4, :])
            cs_pool = st.enter_context(tc.tile_pool(name="csp", bufs=1))
            cos_sb = cs_pool.tile([128, S], F32, name="cos_sb")
            sin_sb = cs_pool.tile([128, S], F32, name="sin_sb")
            nc.sync.dma_start(cos_sb[:], cosT.ap())
            nc.gpsimd.tensor_copy(cos_sb[:], cos_sb[:])
            nc.sync.dma_start(sin_sb[:], sinTs.ap())
            nc.gpsimd.tensor_copy(sin_sb[:], sin_sb[:])
            xt_pool = st.enter_context(tc.tile_pool(name="xt1", bufs=2))
            tmp_pool = st.enter_context(tc.tile_pool(name="ropetmp", bufs=4))
            ps_qk = st.enter_context(tc.tile_pool(name="psqk", bufs=4, space="PSUM"))
            for sc in range(8):
                xt_c = xt_pool.tile([128, 16, 256], F32R, tag="xt", name=f"xt1_{sc}")
                nc.sync.dma_start(xt_c[:], xT3[:, :, 256 * sc:256 * sc + 256])
                nc.gpsimd.tensor_copy(xt_c[:], xt_c[:])
                sl = slice(256 * sc, 256 * sc + 256)
                for m in range(8):
                    pq = ps_qk.tile([128, 256], F32, tag="psqk", name=f"pq{sc}_{m}")
                    for ko in range(16):
                        nc.tensor.matmul(
                            pq[:], wqk_sb[:, ko, 128 * m:128 * m + 128],
                            xt_c[:, ko, :], start=(ko == 0), stop=(ko == 15),
                        )
                    dest = (qrot[m] if m < 4 else krot[m - 4])[:, sl]
                    t1 = tmp_pool.tile([128, 256], F32, tag="t1", name=f"t1_{sc}_{m}")
                    nc.vector.tensor_mul(t1[:], pq[:], cos_sb[:, sl])
                    t2 = tmp_pool.tile([128, 256], F32, tag="t2", name=f"t2_{sc}_{m}")
                    nc.vector.tensor_mul(t2[0:64, :], pq[64:128, :], sin_sb[0:64, sl])
                    nc.vector.tensor_mul(t2[64:128, :], pq[0:64, :], sin_sb[64:128, sl])
                    nc.vector.tensor_add(dest, t1[:], t2[:])

        # ---- phase A2: v projection (natural layout) ----------------------
        v_pool = ab.enter_context(tc.tile_pool(name="vpool", bufs=1))
        vsb = [v_pool.tile([128, 512], F32R, name=f"v{j}") for j in range(16)]
        with ExitStack() as st:
            wv_pool = st.enter_context(tc.tile_pool(name="wvp", bufs=1))
            wv_sb = wv_pool.tile([128, 16, 512], F32R, name="wv_sb")
            for g in range(4):
                nc.sync.dma_start(wv_sb[:, 4 * g:4 * g + 4, :],
                                  wv3[:, 4 * g:4 * g + 4, :])
                nc.gpsimd.tensor_copy(wv_sb[:, 4 * g:4 * g + 4, :],
                                 wv_sb[:, 4 * g:4 * g + 4, :])
            xt_pool = st.enter_context(tc.tile_pool(name="xt2", bufs=2))
            ps_v = st.enter_context(tc.tile_pool(name="psv", bufs=4, space="PSUM"))
            for sc in range(8):
                xt_c = xt_pool.tile([128, 16, 256], F32R, tag="xt", name=f"xt2_{sc}")
                nc.sync.dma_start(xt_c[:], xT3[:, :, 256 * sc:256 * sc + 256])
                nc.gpsimd.tensor_copy(xt_c[:], xt_c[:])
                for u in range(2):
                    j = 2 * sc + u
                    pv = ps_v.tile([128, 512], F32, tag="psv", name=f"pv{j}")
                    for ko in range(16):
                        nc.tensor.matmul(
                            pv[:], xt_c[:, ko, 128 * u:128 * u + 128],
                            wv_sb[:, ko, :], start=(ko == 0), stop=(ko == 15),
                        )
                    nc.scalar.copy(vsb[j][:], pv[:])

        # ---- phase B: attention (score-transposed flash) ------------------
        with ExitStack() as st:
            p_pool = st.enter_context(tc.tile_pool(name="pp", bufs=3))
            misc = st.enter_context(tc.tile_pool(name="miscb", bufs=2))
            sc_ps = st.enter_context(tc.tile_pool(name="scps", bufs=2, space="PSUM"))
            ctx_ps = st.enter_context(tc.tile_pool(name="ctxps", bufs=1, space="PSUM"))
            l_ps = st.enter_context(tc.tile_pool(name="lps", bufs=1, space="PSUM"))
            for sb in range(2):
                for h in range(HLOC):
                    ctx = ctx_ps.tile([128, 1024], F32, tag="ctx", name=f"ctx{sb}_{h}")
                    lps = l_ps.tile([1, 1024], F32, tag="l", name=f"l{sb}_{h}")
                    jmax = 8 * sb + 8
                    for j in range(jmax):
                        dj = j - 8 * sb
                        c0 = max(0, 128 * dj)
                        cks = _chunks(c0)
                        sc_t = sc_ps.tile([128, 1024], F32, tag="scps",
                                          name=f"sc{sb}_{h}_{j}")
                        for (cs, ce) in cks:
                            nc.tensor.matmul(
                                sc_t[:, cs:ce], krot[h][:, 128 * j:128 * j + 128],
                                qrot[h][:, 1024 * sb + cs:1024 * sb + ce],
                                start=True, stop=True,
                            )
                        p_t = p_pool.tile([128, 1024], F32R, tag="p",
                                          name=f"p{sb}_{h}_{j}")
                        nc.scalar.activation(p_t[:, c0:1024], sc_t[:, c0:1024],
                                             AF.Exp, scale=SCALE)
                        if dj >= 0:
                            dsl = slice(128 * dj, 128 * dj + 128)
                            nc.vector.tensor_mul(p_t[:, dsl], p_t[:, dsl], mask_sb[:])
                        last = (j == jmax - 1)
                        for (cs, ce) in cks:
                            nc.tensor.matmul(
                                ctx[:, cs:ce], vsb[j][:, 128 * h:128 * h + 128],
                                p_t[:, cs:ce], start=(j == 0), stop=last,
                                skip_group_check=True,
                            )
                        for (cs, ce) in cks:
                            nc.tensor.matmul(
                                lps[0:1, cs:ce], ones_col[:], p_t[:, cs:ce],
                                start=(j == 0), stop=last, skip_group_check=True,
                            )
                    # normalize: ctxn = ctx * (1/l) broadcast over partitions
                    linv = misc.tile([1, 1024], F32, tag="linv", name=f"li{sb}_{h}")
                    nc.vector.reciprocal_approx_fast(out=linv[:], in_=lps[:])
                    linv_r = misc.tile([1, 1024], F32R, tag="linvr", name=f"lr{sb}_{h}")
                    nc.scalar.copy(linv_r[:], linv[:])
                    bps = sc_ps.tile([128, 1024], F32, tag="scps", name=f"b{sb}_{h}")
                    nc.tensor.matmul(bps[:, 0:512], ones_row[:], linv_r[0:1, 0:512],
                                     start=True, stop=True)
                    nc.tensor.matmul(bps[:, 512:1024], ones_row[:],
                                     linv_r[0:1, 512:1024], start=True, stop=True)
                    bsb = misc.tile([128, 1024], F32, tag="bsb", name=f"bs{sb}_{h}")
                    nc.scalar.copy(bsb[:], bps[:])
                    ctxn = misc.tile([128, 1024], F32R, tag="ctxn", name=f"cn{sb}_{h}")
                    nc.vector.tensor_mul(ctxn[:], ctx[:], bsb[:])
                    nc.sync.dma_start(
                        agin[sb][128 * h:128 * h + 128, :], ctxn[:]
                    )
                nc.gpsimd.collective_compute(
                    "AllGather", mybir.AluOpType.bypass,
                    ins=[agin[sb][:]], outs=[agout[sb][:]],
                    replica_groups=[[0, 1, 2, 3], [4, 5, 6, 7]],
                )

        ab.close()  # free qrot/krot and v before phase C

        # ---- phase C: output projection (512-col slice, K = all heads) ----
        with ExitStack() as st:
            wo_pool = st.enter_context(tc.tile_pool(name="wop", bufs=1))
            wo_sb = wo_pool.tile([128, 16, 512], F32R, name="wo_sb")
            for g in range(4):
                nc.sync.dma_start(wo_sb[:, 4 * g:4 * g + 4, :],
                                  wo3[:, 4 * g:4 * g + 4, :])
                nc.gpsimd.tensor_copy(wo_sb[:, 4 * g:4 * g + 4, :],
                                 wo_sb[:, 4 * g:4 * g + 4, :])
            cg_pool = st.enter_context(tc.tile_pool(name="cgp", bufs=1))
            osb_pool = st.enter_context(tc.tile_pool(name="osbp", bufs=3))
            ps_o = st.enter_context(tc.tile_pool(name="pso", bufs=4, space="PSUM"))
            ctxg = []
            for half in range(2):
                cg = cg_pool.tile([128, 16, 1024], F32R, name=f"ctxg{half}")
                for ko in range(16):
                    nc.sync.dma_start(
                        cg[:, ko, :],
                        agout[half][128 * ko:128 * ko + 128, :],
                    )
                    nc.gpsimd.tensor_copy(cg[:, ko, :], cg[:, ko, :])
                ctxg.append(cg)
            for m in range(16):
                half, mm = m // 8, m % 8
                po = ps_o.tile([128, 512], F32, tag="pso", name=f"po{m}")
                for ko in range(16):
                    nc.tensor.matmul(
                        po[:], ctxg[half][:, ko, 128 * mm:128 * mm + 128],
                        wo_sb[:, ko, :], start=(ko == 0), stop=(ko == 15),
                    )
                osb = osb_pool.tile([128, 512], F32, tag="osb", name=f"osb{m}")
                nc.scalar.copy(osb[:], po[:])
                nc.sync.dma_start(out.ap()[128 * m:128 * m + 128, :], osb[:])

    nc.compile()
    return nc


def _get_runner():
    """Build (once) a persistent jitted SPMD executor for the kernel program."""
    if "runner" in _STATE:
        return _STATE["runner"]
    import jax
    from jax.sharding import Mesh, PartitionSpec
    from jax.experimental.shard_map import shard_map
    from concourse import bass2jax

    nc = _build()
    bass2jax.install_neuronx_cc_hook()

    in_names, out_names, out_avals = [], [], []
    for alloc in nc.m.functions[0].allocations:
        if not isinstance(alloc, mybir.MemoryLocationSet):
            continue
        name = alloc.memorylocations[0].name
        pname = nc.partition_id_tensor.name if nc.partition_id_tensor else None
        if alloc.kind == "ExternalInput":
            if name != pname:
                in_names.append(name)
        elif alloc.kind == "ExternalOutput":
            out_names.append(name)
            out_avals.append(
                jax.core.ShapedArray(tuple(alloc.tensor_shape),
                                     mybir.dt.np(alloc.dtype))
            )
    n_params = len(in_names)
    all_in = list(in_names) + list(out_names)
    pname = nc.partition_id_tensor.name if nc.partition_id_tensor else None
    if pname is not None:
        all_in.append(pname)

    def _body(*args):
        operands = list(args)
        if pname is not None:
            operands.append(bass2jax.partition_id_tensor())
        outs = bass2jax._bass_exec_p.bind(
            *operands,
            out_avals=tuple(out_avals),
            in_names=tuple(all_in),
            out_names=tuple(out_names),
            lowering_input_output_aliases=(),
            sim_require_finite=False,
            sim_require_nnan=False,
            nc=nc,
        )
        return tuple(outs)

    devices = jax.devices()[:NCORES]
    mesh = Mesh(np.asarray(devices), ("core",))
    specs = (PartitionSpec("core"),)
    sharded = jax.jit(
        shard_map(
            _body, mesh=mesh,
            in_specs=specs * (n_params + len(out_names)),
            out_specs=specs * len(out_names),
            check_rep=False,
        ),
        keep_unused=True,
    )
    runner = {
        "fn": sharded, "in_names": in_names, "out_names": out_names,
        "out_avals": out_avals, "n_params": n_params,
    }
    _STATE["runner"] = runner
    return runner


def _prep_inputs(x, cos, sin, w_qkv, w_o):
    """Host-side sharding: per-core input dict list."""
    x = np.asarray(x, dtype=np.float32)
    cos = np.asarray(cos, dtype=np.float32)
    sin = np.asarray(sin, dtype=np.float32)
    w_qkv = np.asarray(w_qkv, dtype=np.float32)
    w_o = np.asarray(w_o, dtype=np.float32)

    cosT = np.ascontiguousarray(cos.T)                      # [128, S]
    sinT = sin.T
    sinTs = np.ascontiguousarray(
        np.concatenate([-sinT[0:64], sinT[64:128]], axis=0))
    pp, ff = np.meshgrid(np.arange(128), np.arange(128), indexing="ij")
    trimask = (pp <= ff).astype(np.float32)                 # t <= s

    in_maps = []
    for c in range(NCORES):
        b, tp = c // TPDEG, c % TPDEG
        cs = 512 * tp
        xT = np.ascontiguousarray(x[b].T)                   # [D, S]
        wq = w_qkv[:, cs:cs + 512]
        wk = w_qkv[:, D + cs:D + cs + 512]
        wqk = np.ascontiguousarray(np.concatenate([wq, wk], axis=1))
        wvs = np.ascontiguousarray(w_qkv[:, 2 * D + cs:2 * D + cs + 512])
        wos = np.ascontiguousarray(w_o[:, cs:cs + 512])
        in_maps.append({
            "xT": xT, "wqk": wqk, "wv": wvs, "wo": wos,
            "cosT": cosT, "sinTs": sinTs, "trimask": trimask,
        })
    return in_maps


def _run(in_maps):
    import jax
    r = _get_runner()
    concat = [
        np.concatenate([np.asarray(in_maps[c][n]) for c in range(NCORES)], axis=0)
        for n in r["in_names"]
    ]
    zeros = [
        np.zeros((NCORES * a.shape[0],) + tuple(a.shape[1:]), a.dtype)
        for a in r["out_avals"]
    ]
    outs = r["fn"](*concat, *zeros)
    outs = [np.asarray(o) for o in jax.block_until_ready(outs)]
    per_core = []
    for c in range(NCORES):
        d = {}
        for i, n in enumerate(r["out_names"]):
            shp = r["out_avals"][i].shape
            d[n] = outs[i].reshape((NCORES,) + shp)[c]
        per_core.append(d)
    return per_core


def kernel(x, cos, sin, w_qkv, w_o):
    in_maps = _prep_inputs(x, cos, sin, w_qkv, w_o)
    results = _run(in_maps)
    B = x.shape[0]
    out = np.empty((B, S, D), dtype=np.float32)
    for c in range(NCORES):
        b, tp = c // TPDEG, c % TPDEG
        out[b, :, 512 * tp:512 * tp + 512] = results[c]["out"]
    return out


# revision 10
# speedup vs baseline: 6.9564x; 6.9564x over previous
"""Trainium2 Bass kernel for nn_Attention_84756884619871.

Causal multi-head attention (B=2, S=2048, D=2048, H=16, Dh=128) with RoPE,
fused QKV projection and output projection.

Sharding (8 NeuronCores): data-parallel over batch (2 groups) x
tensor-parallel over heads (4 cores/group, 4 heads each).  Each core:
  - computes q^T,k^T (transposed layout, RoPE applied) and v for its heads
  - flash-style attention in score-transposed space (p^T[t,s]); softmax
    denominator via ones-vector matmul; no max-subtraction (scores are
    small: exp is safe in fp32)
  - normalized ctx^T shards are AllGather'd over the 4-core group (split in
    two halves along s so the first AG overlaps the second half's compute)
  - each core computes a 512-column slice of the output projection
All matmuls run in float32r (~1.5e-4 rel err, full PE rate at N>=256).
Host assembles the full [2,2048,2048] output from the 8 column slices.
"""

import numpy as np
import ml_dtypes

import concourse.bass as bass
import concourse.tile as tile
import concourse.mybir as mybir
from concourse import bacc
from contextlib import ExitStack

F32 = mybir.dt.float32
F32R = mybir.dt.bfloat16  # matmul operand dtype (bf16: fast ldweights path)
AF = mybir.ActivationFunctionType

D = 2048
S = 2048
NCORES = 8
TPDEG = 4          # tensor-parallel group size (heads)
HLOC = 4           # heads per core
DH = 128
SCALE = float(1.0 / np.sqrt(DH))

_STATE: dict = {}


def _chunks(c0):
    """Split columns [c0, 1024) into <=512 pieces, each >=256 when possible."""
    if c0 == 0:
        return [(0, 512), (512, 1024)]
    if c0 <= 256:
        return [(c0, 512), (512, 1024)]
    if c0 == 384:
        return [(384, 512), (512, 1024)]
    if c0 in (512, 640, 768, 896):
        return [(c0, 1024)]
    raise ValueError(c0)


def _build():
    nc = bacc.Bacc("TRN2", target_bir_lowering=False, debug=False, num_devices=NCORES)
    xT = nc.dram_tensor("xT", [D, S], F32R, kind="ExternalInput")
    wqk = nc.dram_tensor("wqk", [D, 1024], F32R, kind="ExternalInput")
    wv = nc.dram_tensor("wv", [D, 512], F32R, kind="ExternalInput")
    wo = nc.dram_tensor("wo", [D, 512], F32R, kind="ExternalInput")
    cosT = nc.dram_tensor("cosT", [128, S], F32, kind="ExternalInput")
    sinTs = nc.dram_tensor("sinTs", [128, S], F32, kind="ExternalInput")
    trimask = nc.dram_tensor("trimask", [128, 128], F32R, kind="ExternalInput")
    out = nc.dram_tensor("out", [S, 512], F32, kind="ExternalOutput")

    xT3 = xT.ap().rearrange("(ko ki) s -> ki ko s", ki=128)
    wqk3 = wqk.ap().rearrange("(ko ki) c -> ki ko c", ki=128)
    wv3 = wv.ap().rearrange("(ko ki) c -> ki ko c", ki=128)
    wo3 = wo.ap().rearrange("(ko ki) c -> ki ko c", ki=128)

    with tile.TileContext(nc) as tc, ExitStack() as top:
        # ---- persistent tensors -------------------------------------------
        per = top.enter_context(tc.tile_pool(name="persist", bufs=1))
        mask_sb = per.tile([128, 128], F32R, name="mask")
        nc.sync.dma_start(mask_sb[:], trimask.ap())
        nc.gpsimd.tensor_copy(mask_sb[:], mask_sb[:])
        ones_c0 = per.tile([128, 1], F32, name="ones_c0")
        nc.vector.memset(ones_c0[:], 1.0)
        ones_col = per.tile([128, 1], F32R, name="ones_col")
        nc.vector.tensor_copy(ones_col[:], ones_c0[:])
        ones_r0 = per.tile([1, 128], F32, name="ones_r0")
        nc.vector.memset(ones_r0[:], 1.0)
        ones_row = per.tile([1, 128], F32R, name="ones_row")
        nc.vector.tensor_copy(ones_row[:], ones_r0[:])

        dram = top.enter_context(tc.tile_pool(name="dram", bufs=1, space="DRAM"))
        agin = [dram.tile([HLOC * 128, 1024], F32R, name=f"agin{sb}") for sb in range(2)]
        agout = [dram.tile([D, 1024], F32R, name=f"agout{sb}") for sb in range(2)]

        ab = top.enter_context(ExitStack())
        qk_pool = ab.enter_context(tc.tile_pool(name="qkpool", bufs=1))
        qrot = [qk_pool.tile([128, S], F32R, name=f"qrot{h}") for h in range(HLOC)]
        krot = [qk_pool.tile([128, S], F32R, name=f"krot{h}") for h in range(HLOC)]

        # ---- phase A1: q^T,k^T projection + RoPE --------------------------
        with ExitStack() as st:
            wqk_pool = st.enter_context(tc.tile_pool(name="wqkp", bufs=1))
            wqk_sb = wqk_pool.tile([128, 16, 1024], F32R, name="wqk_sb")
            for g in range(4):
                nc.sync.dma_start(wqk_sb[:, 4 * g:4 * g + 4, :],
                                  wqk3[:, 4 * g:4 * g + 4, :])
                nc.gpsimd.tensor_copy(wqk_sb[:, 4 * g:4 * g + 4, :],
                                 wqk_sb[:, 4 * g:4 * g + 4, :])
            cs_pool = st.enter_context(tc.tile_pool(name="csp", bufs=1))
            cos_sb = cs_pool.tile([128, S], F32, name="cos_sb")
            sin_sb = cs_pool.tile([128, S], F32, name="sin_sb")
            nc.sync.dma_start(cos_sb[:], cosT.ap())
            nc.gpsimd.tensor_copy(cos_sb[:], cos_sb[:])
            nc.sync.dma_start(sin_sb[:], sinTs.ap())
            nc.gpsimd.tensor_copy(sin_sb[:], sin_sb[:])
            xt_pool = st.enter_context(tc.tile_pool(name="xt1", bufs=2))
            tmp_pool = st.enter_context(tc.tile_pool(name="ropetmp", bufs=4))
            ps_qk = st.enter_context(tc.tile_pool(name="psqk", bufs=4, space="PSUM"))
            for sc in range(8):
                xt_c = xt_pool.tile([128, 16, 256], F32R, tag="xt", name=f"xt1_{sc}")
                nc.sync.dma_start(xt_c[:], xT3[:, :, 256 * sc:256 * sc + 256])
                nc.gpsimd.tensor_copy(xt_c[:], xt_c[:])
                sl = slice(256 * sc, 256 * sc + 256)
                for m in range(8):
                    pq = ps_qk.tile([128, 256], F32, tag="psqk", name=f"pq{sc}_{m}")
                    for ko in range(16):
                        nc.tensor.matmul(
                            pq[:], wqk_sb[:, ko, 128 * m:128 * m + 128],
                            xt_c[:, ko, :], start=(ko == 0), stop=(ko == 15),
                        )
                    dest = (qrot[m] if m < 4 else krot[m - 4])[:, sl]
                    t1 = tmp_pool.tile([128, 256], F32, tag="t1", name=f"t1_{sc}_{m}")
                    nc.vector.tensor_mul(t1[:], pq[:], cos_sb[:, sl])
                    t2 = tmp_pool.tile([128, 256], F32, tag="t2", name=f"t2_{sc}_{m}")
                    nc.vector.tensor_mul(t2[0:64, :], pq[64:128, :], sin_sb[0:64, sl])
                    nc.vector.tensor_mul(t2[64:128, :], pq[0:64, :], sin_sb[64:128, sl])
                    nc.vector.tensor_add(dest, t1[:], t2[:])

        # ---- phase A2: v projection (natural layout) ----------------------
        v_pool = ab.enter_context(tc.tile_pool(name="vpool", bufs=1))
        vsb = [v_pool.tile([128, 512], F32R, name=f"v{j}") for j in range(16)]
        with ExitStack() as st:
            wv_pool = st.enter_context(tc.tile_pool(name="wvp", bufs=1))
            wv_sb = wv_pool.tile([128, 16, 512], F32R, name="wv_sb")
            for g in range(4):
                nc.sync.dma_start(wv_sb[:, 4 * g:4 * g + 4, :],
                                  wv3[:, 4 * g:4 * g + 4, :])
                nc.gpsimd.tensor_copy(wv_sb[:, 4 * g:4 * g + 4, :],
                                 wv_sb[:, 4 * g:4 * g + 4, :])
            xt_pool = st.enter_context(tc.tile_pool(name="xt2", bufs=2))
            ps_v = st.enter_context(tc.tile_pool(name="psv", bufs=4, space="PSUM"))
            for sc in range(8):
                xt_c = xt_pool.tile([128, 16, 256], F32R, tag="xt", name=f"xt2_{sc}")
                nc.sync.dma_start(xt_c[:], xT3[:, :, 256 * sc:256 * sc + 256])
                nc.gpsimd.tensor_copy(xt_c[:], xt_c[:])
                for u in range(2):
                    j = 2 * sc + u
                    pv = ps_v.tile([128, 512], F32, tag="psv", name=f"pv{j}")
                    for ko in range(16):
                        nc.tensor.matmul(
                            pv[:], xt_c[:, ko, 128 * u:128 * u + 128],
                            wv_sb[:, ko, :], start=(ko == 0), stop=(ko == 15),
                        )
                    nc.scalar.copy(vsb[j][:], pv[:])

        # ---- phase B: attention (score-transposed flash) ------------------
        with ExitStack() as st:
            p_pool = st.enter_context(tc.tile_pool(name="pp", bufs=3))
            misc = st.enter_context(tc.tile_pool(name="miscb", bufs=2))
            sc_ps = st.enter_context(tc.tile_pool(name="scps", bufs=2, space="PSUM"))
            ctx_ps = st.enter_context(tc.tile_pool(name="ctxps", bufs=1, space="PSUM"))
            l_ps = st.enter_context(tc.tile_pool(name="lps", bufs=1, space="PSUM"))
            for sb in range(2):
                for h in range(HLOC):
                    ctx = ctx_ps.tile([128, 1024], F32, tag="ctx", name=f"ctx{sb}_{h}")
                    lps = l_ps.tile([1, 1024], F32, tag="l", name=f"l{sb}_{h}")
                    jmax = 8 * sb + 8
                    for j in range(jmax):
                        dj = j - 8 * sb
                        c0 = max(0, 128 * dj)
                        cks = _chunks(c0)
                        sc_t = sc_ps.tile([128, 1024], F32, tag="scps",
                                          name=f"sc{sb}_{h}_{j}")
                        for (cs, ce) in cks:
                            nc.tensor.matmul(
                                sc_t[:, cs:ce], krot[h][:, 128 * j:128 * j + 128],
                                qrot[h][:, 1024 * sb + cs:1024 * sb + ce],
                                start=True, stop=True,
                            )
                        p_t = p_pool.tile([128, 1024], F32R, tag="p",
                                          name=f"p{sb}_{h}_{j}")
                        nc.scalar.activation(p_t[:, c0:1024], sc_t[:, c0:1024],
                                             AF.Exp, scale=SCALE)
                        if dj >= 0:
                            dsl = slice(128 * dj, 128 * dj + 128)
                            nc.vector.tensor_mul(p_t[:, dsl], p_t[:, dsl], mask_sb[:])
                        last = (j == jmax - 1)
                        for (cs, ce) in cks:
                            nc.tensor.matmul(
                                ctx[:, cs:ce], vsb[j][:, 128 * h:128 * h + 128],
                                p_t[:, cs:ce], start=(j == 0), stop=last,
                                skip_group_check=True,
                            )
                        for (cs, ce) in cks:
                            nc.tensor.matmul(
                                lps[0:1, cs:ce], ones_col[:], p_t[:, cs:ce],
                                start=(j == 0), stop=last, skip_group_check=True,
                            )
                    # normalize: ctxn = ctx * (1/l) broadcast over partitions
                    linv = misc.tile([1, 1024], F32, tag="linv", name=f"li{sb}_{h}")
                    nc.vector.reciprocal_approx_fast(out=linv[:], in_=lps[:])
                    linv_r = misc.tile([1, 1024], F32R, tag="linvr", name=f"lr{sb}_{h}")
                    nc.scalar.copy(linv_r[:], linv[:])
                    bps = sc_ps.tile([128, 1024], F32, tag="scps", name=f"b{sb}_{h}")
                    nc.tensor.matmul(bps[:, 0:512], ones_row[:], linv_r[0:1, 0:512],
                                     start=True, stop=True)
                    nc.tensor.matmul(bps[:, 512:1024], ones_row[:],
                                     linv_r[0:1, 512:1024], start=True, stop=True)
                    bsb = misc.tile([128, 1024], F32, tag="bsb", name=f"bs{sb}_{h}")
                    nc.scalar.copy(bsb[:], bps[:])
                    ctxn = misc.tile([128, 1024], F32R, tag="ctxn", name=f"cn{sb}_{h}")
                    nc.vector.tensor_mul(ctxn[:], ctx[:], bsb[:])
                    nc.sync.dma_start(
                        agin[sb][128 * h:128 * h + 128, :], ctxn[:]
                    )
                nc.gpsimd.collective_compute(
                    "AllGather", mybir.AluOpType.bypass,
                    ins=[agin[sb][:]], outs=[agout[sb][:]],
                    replica_groups=[[0, 1, 2, 3], [4, 5, 6, 7]],
                )

        ab.close()  # free qrot/krot and v before phase C

        # ---- phase C: output projection (512-col slice, K = all heads) ----
        with ExitStack() as st:
            wo_pool = st.enter_context(tc.tile_pool(name="wop", bufs=1))
            wo_sb = wo_pool.tile([128, 16, 512], F32R, name="wo_sb")
            for g in range(4):
                nc.sync.dma_start(wo_sb[:, 4 * g:4 * g + 4, :],
                                  wo3[:, 4 * g:4 * g + 4, :])
                nc.gpsimd.tensor_copy(wo_sb[:, 4 * g:4 * g + 4, :],
                                 wo_sb[:, 4 * g:4 * g + 4, :])
            cg_pool = st.enter_context(tc.tile_pool(name="cgp", bufs=1))
            osb_pool = st.enter_context(tc.tile_pool(name="osbp", bufs=3))
            ps_o = st.enter_context(tc.tile_pool(name="pso", bufs=4, space="PSUM"))
            ctxg = []
            for half in range(2):
                cg = cg_pool.tile([128, 16, 1024], F32R, name=f"ctxg{half}")
                for ko in range(16):
                    nc.sync.dma_start(
                        cg[:, ko, :],
                        agout[half][128 * ko:128 * ko + 128, :],
                    )
                    nc.gpsimd.tensor_copy(cg[:, ko, :], cg[:, ko, :])
                ctxg.append(cg)
            for m in range(16):
                half, mm = m // 8, m % 8
                po = ps_o.tile([128, 512], F32, tag="pso", name=f"po{m}")
                for ko in range(16):
                    nc.tensor.matmul(
                        po[:], ctxg[half][:, ko, 128 * mm:128 * mm + 128],
                        wo_sb[:, ko, :], start=(ko == 0), stop=(ko == 15),
                    )
                osb = osb_pool.tile([128, 512], F32, tag="osb", name=f"osb{m}")
                nc.scalar.copy(osb[:], po[:])
                nc.sync.dma_start(out.ap()[128 * m:128 * m + 128, :], osb[:])

    nc.compile()
    return nc


def _get_runner():
    """Build (once) a persistent jitted SPMD executor for the kernel program."""
    if "runner" in _STATE:
        return _STATE["runner"]
    import jax
    from jax.sharding import Mesh, PartitionSpec
    from jax.experimental.shard_map import shard_map
    from concourse import bass2jax

    nc = _build()
    bass2jax.install_neuronx_cc_hook()

    in_names, out_names, out_avals = [], [], []
    for alloc in nc.m.functions[0].allocations:
        if not isinstance(alloc, mybir.MemoryLocationSet):
            continue
        name = alloc.memorylocations[0].name
        pname = nc.partition_id_tensor.name if nc.partition_id_tensor else None
        if alloc.kind == "ExternalInput":
            if name != pname:
                in_names.append(name)
        elif alloc.kind == "ExternalOutput":
            out_names.append(name)
            out_avals.append(
                jax.core.ShapedArray(tuple(alloc.tensor_shape),
                                     mybir.dt.np(alloc.dtype))
            )
    n_params = len(in_names)
    all_in = list(in_names) + list(out_names)
    pname = nc.partition_id_tensor.name if nc.partition_id_tensor else None
    if pname is not None:
        all_in.append(pname)

    def _body(*args):
        operands = list(args)
        if pname is not None:
            operands.append(bass2jax.partition_id_tensor())
        outs = bass2jax._bass_exec_p.bind(
            *operands,
            out_avals=tuple(out_avals),
            in_names=tuple(all_in),
            out_names=tuple(out_names),
            lowering_input_output_aliases=(),
            sim_require_finite=False,
            sim_require_nnan=False,
            nc=nc,
        )
        return tuple(outs)

    devices = jax.devices()[:NCORES]
    mesh = Mesh(np.asarray(devices), ("core",))
    specs = (PartitionSpec("core"),)
    sharded = jax.jit(
        shard_map(
            _body, mesh=mesh,
            in_specs=specs * (n_params + len(out_names)),
            out_specs=specs * len(out_names),
            check_rep=False,
        ),
        keep_unused=True,
    )
    runner = {
        "fn": sharded, "in_names": in_names, "out_names": out_names,
        "out_avals": out_avals, "n_params": n_params,
    }
    _STATE["runner"] = runner
    return runner


def _prep_inputs(x, cos, sin, w_qkv, w_o):
    """Host-side sharding: per-core input dict list."""
    x = np.asarray(x, dtype=np.float32)
    cos = np.asarray(cos, dtype=np.float32)
    sin = np.asarray(sin, dtype=np.float32)
    w_qkv = np.asarray(w_qkv, dtype=np.float32)
    w_o = np.asarray(w_o, dtype=np.float32)

    cosT = np.ascontiguousarray(cos.T)                      # [128, S]
    sinT = sin.T
    sinTs = np.ascontiguousarray(
        np.concatenate([-sinT[0:64], sinT[64:128]], axis=0))
    pp, ff = np.meshgrid(np.arange(128), np.arange(128), indexing="ij")
    trimask = (pp <= ff).astype(np.float32)                 # t <= s

    in_maps = []
    for c in range(NCORES):
        b, tp = c // TPDEG, c % TPDEG
        cs = 512 * tp
        xT = np.ascontiguousarray(x[b].T)                   # [D, S]
        wq = w_qkv[:, cs:cs + 512]
        wk = w_qkv[:, D + cs:D + cs + 512]
        wqk = np.ascontiguousarray(np.concatenate([wq, wk], axis=1))
        wvs = np.ascontiguousarray(w_qkv[:, 2 * D + cs:2 * D + cs + 512])
        wos = np.ascontiguousarray(w_o[:, cs:cs + 512])
        bf = ml_dtypes.bfloat16
        in_maps.append({
            "xT": xT.astype(bf), "wqk": wqk.astype(bf), "wv": wvs.astype(bf),
            "wo": wos.astype(bf),
            "cosT": cosT, "sinTs": sinTs, "trimask": trimask.astype(bf),
        })
    return in_maps


def _run(in_maps):
    import jax
    r = _get_runner()
    concat = [
        np.concatenate([np.asarray(in_maps[c][n]) for c in range(NCORES)], axis=0)
        for n in r["in_names"]
    ]
    zeros = [
        np.zeros((NCORES * a.shape[0],) + tuple(a.shape[1:]), a.dtype)
        for a in r["out_avals"]
    ]
    outs = r["fn"](*concat, *zeros)
    outs = [np.asarray(o) for o in jax.block_until_ready(outs)]
    per_core = []
    for c in range(NCORES):
        d = {}
        for i, n in enumerate(r["out_names"]):
            shp = r["out_avals"][i].shape
            d[n] = outs[i].reshape((NCORES,) + shp)[c]
        per_core.append(d)
    return per_core


def kernel(x, cos, sin, w_qkv, w_o):
    in_maps = _prep_inputs(x, cos, sin, w_qkv, w_o)
    results = _run(in_maps)
    B = x.shape[0]
    out = np.empty((B, S, D), dtype=np.float32)
    for c in range(NCORES):
        b, tp = c // TPDEG, c % TPDEG
        out[b, :, 512 * tp:512 * tp + 512] = results[c]["out"]
    return out
